# revision 4
# baseline (speedup 1.0000x reference)
"""MultiHeadAttention Trainium2 kernel (8 NeuronCores, SPMD).

Sharding: data-parallel over batch (B=2), tensor-parallel over heads
(16 heads -> 4 per core).  Core c handles batch b=c//4, head group
g=c%4 (heads 4g..4g+3).  Wq/Wk/Wv are split column-wise, Wo row-wise;
the per-core Wo partial outputs are summed on the host (replaces the
all-reduce).

Device dataflow per core (bf16 matmuls, f32 PSUM accumulation):
  qT = Wq_g^T x^T   [256, 2048]   (heads on partitions, dh=64 each)
  kT = Wk_g^T x^T   [256, 2048]
  v  = x Wv_g       [2048, 256] stored interleaved with a ones column
                    per head: vaug[st] = [vA|1|vB|1|vC|1|vD|1]
  per (s_q chunk of 512, head pair):
    logitsT[s_k, s_q] = kT^T qT / 8       (two heads packed in PE row
                                           groups, K=64 each)
    p = exp(logitsT)  on ScalarE, scale=1/8 fused, bf16 out
    accT[65, s_q] += vaug_h^T p           (row 64 = softmax denominator)
    outcatT[h] = accT[0:64] * bcast(1/accT[64])   (deferred softmax norm)
  partial = outcatT^T Wo_g  -> DRAM f32
"""

import sys

import numpy as np

sys.path.insert(0, "/opt/trn_rl_repo")

import ml_dtypes  # noqa: E402

import concourse.bass as bass  # noqa: E402
import concourse.mybir as mybir  # noqa: E402
import concourse.tile as tile  # noqa: E402
from concourse import bacc  # noqa: E402
from concourse.bass import ts  # noqa: E402
from concourse.bass_utils import run_bass_kernel_spmd  # noqa: E402

S = 2048  # sequence length (S * X)
D = 1024  # model dim
H = 16  # total heads
HL = 4  # heads per core
DH = 64  # head dim
DQ = HL * DH  # per-core projection width = 256
NK = D // 128  # K tiles for projections = 8
NST = S // 128  # s_k tiles = 16
NCH = S // 512  # s_q chunks = 4

BF16 = mybir.dt.bfloat16
F32 = mybir.dt.float32

TRACE = False
LAST_RESULTS = None

_BUILT = None


def _emit(ctx, tc, io):
    nc = tc.nc
    xq, xk, xv = io["xqT"], io["xkT"], io["xvT"]
    wq, wk, wv, wo = io["wq"], io["wk"], io["wv"], io["wo"]
    bq, bk, bv = io["bq"], io["bk"], io["bv"]
    out = io["out"]

    consts = ctx.enter_context(tc.tile_pool(name="consts", bufs=1))
    xin = ctx.enter_context(tc.tile_pool(name="xin", bufs=1))
    qk = ctx.enter_context(tc.tile_pool(name="qk", bufs=1))
    ptiles = ctx.enter_context(tc.tile_pool(name="ptiles", bufs=3))
    norm = ctx.enter_context(tc.tile_pool(name="norm", bufs=4))
    osb_pool = ctx.enter_context(tc.tile_pool(name="osb", bufs=3))
    psum_mm = ctx.enter_context(tc.tile_pool(name="psum_mm", bufs=4, space="PSUM"))
    psum_lg = ctx.enter_context(tc.tile_pool(name="psum_lg", bufs=2, space="PSUM"))

    # ---- load weights / biases ----
    wq_t = [consts.tile([128, DQ], BF16, tag=f"wq{k}", name=f"wq{k}") for k in range(NK)]
    wk_t = [consts.tile([128, DQ], BF16, tag=f"wk{k}", name=f"wk{k}") for k in range(NK)]
    wv_t = [consts.tile([128, DQ], BF16, tag=f"wv{k}", name=f"wv{k}") for k in range(NK)]
    for k in range(NK):
        nc.sync.dma_start(wq_t[k][:], wq[ts(k, 128), :])
        nc.sync.dma_start(wk_t[k][:], wk[ts(k, 128), :])
        nc.sync.dma_start(wv_t[k][:], wv[ts(k, 128), :])
    wo_t = [consts.tile([128, D], BF16, tag=f"wo{k}", name=f"wo{k}") for k in range(2)]
    for k in range(2):
        nc.sync.dma_start(wo_t[k][:], wo[ts(k, 128), :])

    # bq/bk as [128, 2] per-partition scalars (col j = dq 128j..128j+127)
    bq_sb = consts.tile([128, 2], F32, tag="bq", name="bq_sb")
    bk_sb = consts.tile([128, 2], F32, tag="bk", name="bk_sb")
    for t, src in ((bq_sb, bq), (bk_sb, bk)):
        nc.gpsimd.dma_start(
            out=t[:], in_=bass.AP(tensor=src.tensor, offset=src.offset, ap=[[1, 128], [128, 2]])
        )
    # bv broadcast to all partitions [128, 256]
    bv_sb = consts.tile([128, DQ], F32, tag="bv", name="bv_sb")
    nc.gpsimd.dma_start(
        out=bv_sb[:], in_=bass.AP(tensor=bv.tensor, offset=bv.offset, ap=[[0, 128], [1, DQ]])
    )

    # ---- load x^T tiles ----
    xq_t = [xin.tile([128, S], BF16, tag=f"xq{k}", name=f"xq{k}") for k in range(NK)]
    xk_t = [xin.tile([128, S], BF16, tag=f"xk{k}", name=f"xk{k}") for k in range(NK)]
    xv_t = [xin.tile([128, S], BF16, tag=f"xv{k}", name=f"xv{k}") for k in range(NK)]
    for k in range(NK):
        nc.sync.dma_start(xq_t[k][:], xq[ts(k, 128), :])
    for k in range(NK):
        nc.sync.dma_start(xk_t[k][:], xk[ts(k, 128), :])
    for k in range(NK):
        nc.sync.dma_start(xv_t[k][:], xv[ts(k, 128), :])

    # ---- projections: qT, kT = [256, 2048] as 2 tiles of [128, 2048] ----
    qT = [qk.tile([128, S], BF16, tag=f"qT{m}", name=f"qT{m}") for m in range(2)]
    kT = [qk.tile([128, S], BF16, tag=f"kT{m}", name=f"kT{m}") for m in range(2)]
    for (w_t, x_t, dst, b_sb) in ((wq_t, xq_t, qT, bq_sb), (wk_t, xk_t, kT, bk_sb)):
        for m in range(2):
            for c in range(NCH):
                ps = psum_mm.tile([128, 512], F32, tag="mm", name="mm")
                for k in range(NK):
                    nc.tensor.matmul(
                        ps[:],
                        w_t[k][:, ts(m, 128)],
                        x_t[k][:, ts(c, 512)],
                        start=(k == 0),
                        stop=(k == NK - 1),
                    )
                nc.vector.tensor_scalar_add(
                    dst[m][:, ts(c, 512)], ps[:], b_sb[:, m : m + 1]
                )

    # ---- v projection -> vaug[st] = [vA|1|vB|1|vC|1|vD|1]  [128, 260] ----
    vaug = [qk.tile([128, HL * (DH + 1)], BF16, tag=f"vaug{st}", name=f"vaug{st}") for st in range(NST)]
    for st in range(NST):
        ps = psum_mm.tile([128, DQ], F32, tag="mm", name="mm")
        for k in range(NK):
            nc.tensor.matmul(
                ps[:],
                xv_t[k][:, ts(st, 128)],
                wv_t[k][:],
                start=(k == 0),
                stop=(k == NK - 1),
            )
        for h in range(HL):
            nc.vector.tensor_add(
                vaug[st][:, h * 65 : h * 65 + 64],
                ps[:, ts(h, DH)],
                bv_sb[:, ts(h, DH)],
            )
            nc.vector.memset(vaug[st][:, h * 65 + 64 : h * 65 + 65], 1.0)

    # ---- outcatT [256, 2048] bf16 (normalized attention output^T) ----
    octT = [qk.tile([128, S], BF16, tag=f"octT{m}", name=f"octT{m}") for m in range(2)]

    for c in range(NCH):
        for pr in range(2):  # head pair (2pr, 2pr+1); kT/qT tile index = pr
            acc = [psum_mm.tile([65, 512], F32, tag="mm", name="mm") for _ in range(2)]
            pending = None  # (st, p_tile) awaiting accumulation matmuls
            for st in range(NST):
                lg = psum_lg.tile([128, 1024], F32, tag="lg", name="lg")
                for hh in range(2):
                    nc.tensor.matmul(
                        lg[:, ts(hh, 512)],
                        kT[pr][ts(hh, 64), ts(st, 128)],
                        qT[pr][ts(hh, 64), ts(c, 512)],
                        start=True,
                        stop=True,
                    )
                p = ptiles.tile([128, 1024], BF16, tag="p", name="p")
                nc.scalar.activation(
                    p[:], lg[:], mybir.ActivationFunctionType.Exp, scale=0.125
                )
                if pending is not None:
                    pst, pp = pending
                    for hh in range(2):
                        h = 2 * pr + hh
                        nc.tensor.matmul(
                            acc[hh][:],
                            vaug[pst][:, h * 65 : h * 65 + 65],
                            pp[:, ts(hh, 512)],
                            start=(pst == 0),
                            stop=(pst == NST - 1),
                        )
                pending = (st, p)
            pst, pp = pending
            for hh in range(2):
                h = 2 * pr + hh
                nc.tensor.matmul(
                    acc[hh][:],
                    vaug[pst][:, h * 65 : h * 65 + 65],
                    pp[:, ts(hh, 512)],
                    start=(pst == 0),
                    stop=(pst == NST - 1),
                )
            # normalize: octT[pr][64*hh.., chunk] = acc[0:64] / acc[64]
            for hh in range(2):
                recip = norm.tile([1, 512], F32, tag="recip", name="recip")
                nc.vector.reciprocal(recip[:], acc[hh][64:65, :])
                bc = norm.tile([64, 512], F32, tag="bcast", name="bcast")
                nc.gpsimd.partition_broadcast(bc[:], recip[:])
                nc.vector.tensor_mul(
                    octT[pr][ts(hh, 64), ts(c, 512)], acc[hh][0:64, :], bc[:]
                )

        # ---- output projection for this chunk ----
        for smt in range(4):
            row = c * 512 + smt * 128
            for ncho in range(2):
                ps = psum_mm.tile([128, 512], F32, tag="mm", name="mm")
                for k in range(2):
                    nc.tensor.matmul(
                        ps[:],
                        octT[k][:, row : row + 128],
                        wo_t[k][:, ts(ncho, 512)],
                        start=(k == 0),
                        stop=(k == 1),
                    )
                osb = osb_pool.tile([128, 512], F32, tag="osb", name="osb")
                nc.vector.tensor_copy(osb[:], ps[:])
                nc.sync.dma_start(out[row : row + 128, ts(ncho, 512)], osb[:])


def _build():
    global _BUILT
    if _BUILT is not None:
        return _BUILT
    nc = bacc.Bacc(
        "TRN2",
        target_bir_lowering=False,
        debug=False,
        enable_asserts=False,
        num_devices=8,
    )
    io = {}
    io["xqT"] = nc.dram_tensor("xqT", [D, S], BF16, kind="ExternalInput").ap()
    io["xkT"] = nc.dram_tensor("xkT", [D, S], BF16, kind="ExternalInput").ap()
    io["xvT"] = nc.dram_tensor("xvT", [D, S], BF16, kind="ExternalInput").ap()
    io["wq"] = nc.dram_tensor("wq", [D, DQ], BF16, kind="ExternalInput").ap()
    io["wk"] = nc.dram_tensor("wk", [D, DQ], BF16, kind="ExternalInput").ap()
    io["wv"] = nc.dram_tensor("wv", [D, DQ], BF16, kind="ExternalInput").ap()
    io["wo"] = nc.dram_tensor("wo", [DQ, D], BF16, kind="ExternalInput").ap()
    io["bq"] = nc.dram_tensor("bq", [DQ], F32, kind="ExternalInput").ap()
    io["bk"] = nc.dram_tensor("bk", [DQ], F32, kind="ExternalInput").ap()
    io["bv"] = nc.dram_tensor("bv", [DQ], F32, kind="ExternalInput").ap()
    io["out"] = nc.dram_tensor("out", [S, D], F32, kind="ExternalOutput").ap()
    from contextlib import ExitStack

    with tile.TileContext(nc) as tc, ExitStack() as ctx:
        _emit(ctx, tc, io)
    nc.compile()
    _BUILT = nc
    return nc


def kernel(**inputs):
    global LAST_RESULTS
    bf16 = ml_dtypes.bfloat16
    query = np.asarray(inputs["query"], np.float32).reshape(2, S, D)
    key = np.asarray(inputs["key"], np.float32).reshape(2, S, D)
    value = np.asarray(inputs["value"], np.float32).reshape(2, S, D)
    Wq = np.asarray(inputs["Wq"], np.float32)
    Wk = np.asarray(inputs["Wk"], np.float32)
    Wv = np.asarray(inputs["Wv"], np.float32)
    Wo = np.asarray(inputs["Wo"], np.float32)
    bq = np.asarray(inputs["bq"], np.float32)
    bk = np.asarray(inputs["bk"], np.float32)
    bv = np.asarray(inputs["bv"], np.float32)
    bo = np.asarray(inputs["bo"], np.float32)

    xT = {}
    for b in range(2):
        xT[("q", b)] = np.ascontiguousarray(query[b].T).astype(bf16)
        xT[("k", b)] = np.ascontiguousarray(key[b].T).astype(bf16)
        xT[("v", b)] = np.ascontiguousarray(value[b].T).astype(bf16)

    in_maps = []
    for c in range(8):
        b, g = c // 4, c % 4
        sl = slice(g * DQ, (g + 1) * DQ)
        in_maps.append(
            {
                "xqT": xT[("q", b)],
                "xkT": xT[("k", b)],
                "xvT": xT[("v", b)],
                "wq": np.ascontiguousarray(Wq[:, sl]).astype(bf16),
                "wk": np.ascontiguousarray(Wk[:, sl]).astype(bf16),
                "wv": np.ascontiguousarray(Wv[:, sl]).astype(bf16),
                "wo": np.ascontiguousarray(Wo[sl, :]).astype(bf16),
                "bq": np.ascontiguousarray(bq[sl]),
                "bk": np.ascontiguousarray(bk[sl]),
                "bv": np.ascontiguousarray(bv[sl]),
            }
        )

    nc = _build()
    res = run_bass_kernel_spmd(
        nc, in_maps, core_ids=list(range(8)), trace=TRACE
    )
    LAST_RESULTS = res

    full = np.zeros((2, S, D), np.float32)
    for c in range(8):
        full[c // 4] += res.results[c]["out"]
    full += bo[None, None, :]
    return full.reshape(2, S, 1, D)
